# revision 1
# baseline (speedup 1.0000x reference)
"""Trainium2 Bass kernel: row-wise cosine similarity discriminator.

Computes, for full inputs s, h_rl, h_fk of shape [B=8, N=8192, D=512] f32:
    out = concat(rowdot(l2n(s), l2n(h_rl)), rowdot(l2n(s), l2n(h_fk)), axis=1)
with l2n(x) = x / max(||x||_2, 1e-12), giving out shape [8, 16384] f32.

Sharding: pure data parallel over batch B — core b processes batch b.

Per-core kernel strategy (memory-bound: 48 MiB input / core):
  - rows on SBUF partitions; 64 row-tiles of [128 rows, 512] f32, loaded
    as 1 MiB HWDGE DMAs (GJ=4 row-tiles per dma_start, 3D access pattern)
  - per group: gpsimd computes the 2 cross products s*h, ACT computes the
    3 squares, and the DVE (bottleneck engine, tensor_reduce has no 2x
    mode) does only the 5 batched [128, GJ, 512] -> [128, GJ] reductions
  - finals (sqrt/clamp/reciprocal/scale) on tiny [128, 64] stats tiles,
    output transposed on the idle PE, stored contiguously
  - this walrus build cannot encode multi-wait Drain/STT instructions or
    ISA-encoded ops (tensor_tensor_reduce, activation accum_out):
    _fix_tail_drain_waits() rewrites multi-wait instructions into
    single-wait EventSemaphores
"""

import numpy as np

import concourse.bass as bass
import concourse.mybir as mybir
import concourse.tile as tile
from concourse.bass_utils import run_bass_kernel_spmd
from concourse.masks import make_identity

B, N, D = 8, 8192, 512


def _fix_tail_drain_waits(nc):
    """This image's walrus cannot encode more than one sem wait on several
    instruction kinds (Tile's end-of-kernel Drain, STT, ...). Move each
    wait of any multi-wait instruction onto its own EventSemaphore
    inserted right before it on the same engine — identical semantics
    (engine program order), always encodable."""
    for fn in nc.m.functions:
        for bb in fn.blocks:
            new = []
            for inst in bb.instructions:
                si = inst.sync_info
                if (
                    not isinstance(inst, mybir.InstEventSemaphore)
                    and si is not None
                    and si.on_wait
                    and len(si.on_wait) > 1
                ):
                    for k, w in enumerate(list(si.on_wait)):
                        ev = mybir.InstEventSemaphore(
                            name=f"{inst.name}-prewait{k}", ins=[], outs=[]
                        )
                        ev.engine = inst.engine
                        ev.sync_info = mybir.SyncInfo(on_wait=[w], on_update=[])
                        new.append(ev)
                    inst.sync_info = mybir.SyncInfo(
                        on_wait=[], on_update=list(si.on_update)
                    )
                new.append(inst)
            bb.instructions[:] = new
P = 128                    # SBUF partitions (rows per tile)
NT = N // P                # 64 row-tiles per core
GJ = 4                     # row-tiles per dma_start (GJ*P*D*4 = 1 MiB)
NG = NT // GJ              # dma groups
EPS = 1e-12
F32 = mybir.dt.float32


def build_nc():
    nc = bass.Bass(trn_type="TRN2")
    s_h = nc.declare_dram_parameter("s", [N, D], F32, isOutput=False)
    hrl_h = nc.declare_dram_parameter("h_rl", [N, D], F32, isOutput=False)
    hfk_h = nc.declare_dram_parameter("h_fk", [N, D], F32, isOutput=False)
    out_h = nc.declare_dram_parameter("out", [2, NT, P], F32, isOutput=True)

    # DRAM view: row r = (g*GJ + j)*P + p  ->  [g, p, j, d]
    def grouped(h):
        return h[:, :].rearrange("(g j p) d -> g p j d", j=GJ, p=P)

    s_g, hrl_g, hfk_g = grouped(s_h), grouped(hrl_h), grouped(hfk_h)

    with tile.TileContext(nc) as tc:
        with (
            tc.tile_pool(name="ins", bufs=2) as ins,
            tc.tile_pool(name="scrp", bufs=4) as scrp,
            tc.tile_pool(name="scrq", bufs=2) as scrq,
            tc.tile_pool(name="stats", bufs=1) as stats,
            tc.tile_pool(name="fin", bufs=1) as fin,
            tc.tile_pool(name="psum", bufs=1, space="PSUM") as psum,
        ):
            # per-row accumulators, column t = global tile index
            # stats_q: [ss, hh_rl, hh_fk]; stats_p: [sp_rl, sp_fk]
            stats_q = stats.tile([P, 3, NT], F32, tag="stats_q")
            stats_p = stats.tile([P, 2, NT], F32, tag="stats_p")
            ss, hh_rl, hh_fk = (stats_q[:, k, :] for k in range(3))
            sp_rl, sp_fk = (stats_p[:, k, :] for k in range(2))

            Sq = mybir.ActivationFunctionType.Square
            Red = dict(axis=mybir.AxisListType.X, op=mybir.AluOpType.add)

            for g in range(NG):
                s_t = ins.tile([P, GJ, D], F32, tag="s")
                h1_t = ins.tile([P, GJ, D], F32, tag="h_rl")
                h2_t = ins.tile([P, GJ, D], F32, tag="h_fk")
                # products: gpsimd (2 ops), squares: ACT (3 ops); the DVE
                # does only the 5 batched reductions (it is the
                # bottleneck engine: tensor_reduce has no 2x mode)
                p1 = scrp.tile([P, GJ, D], F32, tag="p_rl")
                p2 = scrp.tile([P, GJ, D], F32, tag="p_fk")
                q0 = scrq.tile([P, GJ, D], F32, tag="sq_s")
                q1 = scrq.tile([P, GJ, D], F32, tag="sq_rl")
                q2 = scrq.tile([P, GJ, D], F32, tag="sq_fk")
                cols = slice(g * GJ, (g + 1) * GJ)
                terms = ((p1, stats_p, 0), (p2, stats_p, 1), (q0, stats_q, 0),
                         (q1, stats_q, 1), (q2, stats_q, 2))
                if g == 0:
                    # fine-grained first group so the DVE starts sooner
                    for j in range(GJ):
                        nc.sync.dma_start(out=s_t[:, j, :], in_=s_g[0][:, j, :])
                        nc.sync.dma_start(out=h1_t[:, j, :], in_=hrl_g[0][:, j, :])
                        nc.sync.dma_start(out=h2_t[:, j, :], in_=hfk_g[0][:, j, :])
                        nc.scalar.activation(out=q0[:, j, :], in_=s_t[:, j, :], func=Sq)
                        nc.scalar.activation(out=q1[:, j, :], in_=h1_t[:, j, :], func=Sq)
                        nc.scalar.activation(out=q2[:, j, :], in_=h2_t[:, j, :], func=Sq)
                        nc.gpsimd.tensor_tensor(out=p1[:, j, :], in0=s_t[:, j, :],
                                                in1=h1_t[:, j, :], op=mybir.AluOpType.mult)
                        nc.gpsimd.tensor_tensor(out=p2[:, j, :], in0=s_t[:, j, :],
                                                in1=h2_t[:, j, :], op=mybir.AluOpType.mult)
                        jc = slice(j, j + 1)
                        for src, dst, k in terms:
                            nc.vector.tensor_reduce(
                                out=dst[:, k, jc], in_=src[:, j, :], **Red)
                    continue
                nc.sync.dma_start(out=s_t, in_=s_g[g])
                nc.sync.dma_start(out=h1_t, in_=hrl_g[g])
                nc.sync.dma_start(out=h2_t, in_=hfk_g[g])
                nc.gpsimd.tensor_tensor(out=p1, in0=s_t, in1=h1_t,
                                        op=mybir.AluOpType.mult)
                nc.gpsimd.tensor_tensor(out=p2, in0=s_t, in1=h2_t,
                                        op=mybir.AluOpType.mult)
                nc.scalar.activation(out=q0, in_=s_t, func=Sq)
                nc.scalar.activation(out=q1, in_=h1_t, func=Sq)
                nc.scalar.activation(out=q2, in_=h2_t, func=Sq)
                for src, dst, k in terms:
                    nc.vector.tensor_reduce(out=dst[:, k, cols], in_=src, **Red)

            # ---- finals on [P, NT] stats tiles (kept off the DVE except
            # reciprocal, which only the DVE implements accurately) ----
            Sqrt = mybir.ActivationFunctionType.Sqrt
            ns = fin.tile([P, NT], F32, tag="ns")
            n1 = fin.tile([P, NT], F32, tag="n1")
            n2 = fin.tile([P, NT], F32, tag="n2")
            nc.scalar.activation(out=ns, in_=ss, func=Sqrt)
            nc.scalar.activation(out=n1, in_=hh_rl, func=Sqrt)
            nc.scalar.activation(out=n2, in_=hh_fk, func=Sqrt)
            nc.gpsimd.tensor_scalar_max(ns, ns, EPS)
            nc.gpsimd.tensor_scalar_max(n1, n1, EPS)
            nc.gpsimd.tensor_scalar_max(n2, n2, EPS)
            den1 = fin.tile([P, NT], F32, tag="den1")
            den2 = fin.tile([P, NT], F32, tag="den2")
            nc.gpsimd.tensor_tensor(den1, ns, n1, op=mybir.AluOpType.mult)
            nc.gpsimd.tensor_tensor(den2, ns, n2, op=mybir.AluOpType.mult)
            nc.vector.reciprocal(den1, den1)
            nc.vector.reciprocal(den2, den2)
            o1 = fin.tile([P, NT], F32, tag="o1")
            o2 = fin.tile([P, NT], F32, tag="o2")
            nc.gpsimd.tensor_tensor(o1, sp_rl, den1, op=mybir.AluOpType.mult)
            nc.gpsimd.tensor_tensor(o2, sp_fk, den2, op=mybir.AluOpType.mult)

            # transpose [P, NT] -> [NT, P] on the (idle) tensor engine
            ident = fin.tile([P, P], F32, tag="ident")
            make_identity(nc, ident)
            po1 = psum.tile([NT, P], F32, tag="po1")
            po2 = psum.tile([NT, P], F32, tag="po2")
            nc.tensor.transpose(po1, o1, ident)
            nc.tensor.transpose(po2, o2, ident)
            o1t = fin.tile([NT, P], F32, tag="o1t")
            o2t = fin.tile([NT, P], F32, tag="o2t")
            nc.scalar.copy(o1t, po1)
            nc.scalar.copy(o2t, po2)
            nc.sync.dma_start(out=out_h[0], in_=o1t)
            nc.sync.dma_start(out=out_h[1], in_=o2t)

    _fix_tail_drain_waits(nc)
    return nc


_NC_CACHE = None


def kernel(s, h_rl, h_fk, trace=False):
    global _NC_CACHE
    s = np.ascontiguousarray(np.asarray(s, dtype=np.float32))
    h_rl = np.ascontiguousarray(np.asarray(h_rl, dtype=np.float32))
    h_fk = np.ascontiguousarray(np.asarray(h_fk, dtype=np.float32))
    assert s.shape == (B, N, D), s.shape

    if _NC_CACHE is None:
        _NC_CACHE = build_nc()
    nc = _NC_CACHE

    in_maps = [
        {"s": s[b], "h_rl": h_rl[b], "h_fk": h_fk[b]} for b in range(B)
    ]
    res = run_bass_kernel_spmd(nc, in_maps, core_ids=list(range(B)), trace=trace)
    out = np.empty((B, 2 * N), dtype=np.float32)
    for b in range(B):
        o = res.results[b]["out"].reshape(2, N)
        out[b, :N] = o[0]
        out[b, N:] = o[1]
    if trace:
        return out, res
    return out



# revision 2
# speedup vs baseline: 1.1648x; 1.1648x over previous
"""Trainium2 Bass kernel: row-wise cosine similarity discriminator.

Computes, for full inputs s, h_rl, h_fk of shape [B=8, N=8192, D=512] f32:
    out = concat(rowdot(l2n(s), l2n(h_rl)), rowdot(l2n(s), l2n(h_fk)), axis=1)
with l2n(x) = x / max(||x||_2, 1e-12), giving out shape [8, 16384] f32.

Sharding: pure data parallel over batch B — core b processes batch b.

Per-core roofline: 48 MiB of input reads; one HWDGE queue saturates HBM at
~341 GB/s (measured) -> ~147.5 us DMA floor. The previous build was
DVE-bound (5 reduction streams x 1 elem/cycle = ~177 us busy). This build
rebalances so every compute engine sits under the DMA floor (HW-measured
costs per [128, 4x512] f32 group):
  - ACT: norms of s and h_rl via Square+accum_out (one pass squares AND
    row-sums; accum granularity [P,1] forces per-row-tile ops, 799 ns each)
    + batched Square of h_fk -> ~8.3 us/group (132 us)
  - DVE: the two dot reduces + h_fk norm reduce + a 2-row-tile slice of the
    s*h_rl mult -> ~7.7 us/group (124 us)
  - Pool (gpsimd): s*h_fk mult + other half of s*h_rl -> ~6.8 us/group
    (109 us, it is noisy so keep it light)
  - all input DMAs on the single sync HWDGE queue (dual-queue measured no
    faster), 1 MiB per dma_start, 2 KiB descriptor lines
  - tensor_tensor_reduce does not encode on this walrus build (verified);
    activation accum_out does
  - finals (sqrt/clamp/reciprocal/scale) on tiny [128, 64] stats tiles,
    output transposed on the idle PE, stored contiguously
  - _fix_tail_drain_waits() rewrites multi-wait instructions into
    single-wait EventSemaphores (this walrus build cannot encode multi-wait
    Drain/STT instructions)
"""

import numpy as np

import concourse.bass as bass
import concourse.mybir as mybir
import concourse.tile as tile
from concourse.bass_utils import run_bass_kernel_spmd
from concourse.masks import make_identity

B, N, D = 8, 8192, 512


def _fix_tail_drain_waits(nc):
    """This image's walrus cannot encode more than one sem wait on several
    instruction kinds (Tile's end-of-kernel Drain, STT, ...). Move each
    wait of any multi-wait instruction onto its own EventSemaphore
    inserted right before it on the same engine — identical semantics
    (engine program order), always encodable."""
    for fn in nc.m.functions:
        for bb in fn.blocks:
            new = []
            for inst in bb.instructions:
                si = inst.sync_info
                if (
                    not isinstance(inst, mybir.InstEventSemaphore)
                    and si is not None
                    and si.on_wait
                    and len(si.on_wait) > 1
                ):
                    for k, w in enumerate(list(si.on_wait)):
                        ev = mybir.InstEventSemaphore(
                            name=f"{inst.name}-prewait{k}", ins=[], outs=[]
                        )
                        ev.engine = inst.engine
                        ev.sync_info = mybir.SyncInfo(on_wait=[w], on_update=[])
                        new.append(ev)
                    inst.sync_info = mybir.SyncInfo(
                        on_wait=[], on_update=list(si.on_update)
                    )
                new.append(inst)
            bb.instructions[:] = new


P = 128                    # SBUF partitions (rows per tile)
NT = N // P                # 64 row-tiles per core
GJ = 4                     # row-tiles per dma_start (GJ*P*D*4 = 1 MiB)
NG = NT // GJ              # dma groups
EPS = 1e-12
F32 = mybir.dt.float32
BF16 = mybir.dt.bfloat16


def build_nc():
    nc = bass.Bass(trn_type="TRN2")
    s_h = nc.declare_dram_parameter("s", [N, D], F32, isOutput=False)
    hrl_h = nc.declare_dram_parameter("h_rl", [N, D], F32, isOutput=False)
    hfk_h = nc.declare_dram_parameter("h_fk", [N, D], F32, isOutput=False)
    out_h = nc.declare_dram_parameter("out", [2, NT, P], F32, isOutput=True)

    # DRAM view: row r = (g*GJ + j)*P + p  ->  [g, p, j, d]
    def grouped(h):
        return h[:, :].rearrange("(g j p) d -> g p j d", j=GJ, p=P)

    s_g, hrl_g, hfk_g = grouped(s_h), grouped(hrl_h), grouped(hfk_h)

    Sq = mybir.ActivationFunctionType.Square
    Mul = mybir.AluOpType.mult
    Red = dict(axis=mybir.AxisListType.X, op=mybir.AluOpType.add)

    with tile.TileContext(nc) as tc:
        with (
            tc.tile_pool(name="ins", bufs=2) as ins,
            tc.tile_pool(name="scrp", bufs=2) as scrp,
            tc.tile_pool(name="scrq", bufs=2) as scrq,
            tc.tile_pool(name="stats", bufs=1) as stats,
            tc.tile_pool(name="fin", bufs=1) as fin,
            tc.tile_pool(name="psum", bufs=1, space="PSUM") as psum,
        ):
            # per-row accumulators, column t = global row-tile index
            # stats_q: [ss, hh_rl, hh_fk]; stats_p: [sp_rl, sp_fk]
            stats_q = stats.tile([P, 3, NT], F32, tag="stats_q")
            stats_p = stats.tile([P, 2, NT], F32, tag="stats_p")
            ss, hh_rl, hh_fk = (stats_q[:, k, :] for k in range(3))
            sp_rl, sp_fk = (stats_p[:, k, :] for k in range(2))

            for g in range(NG):
                s_t = ins.tile([P, GJ, D], F32, tag="s")
                h1_t = ins.tile([P, GJ, D], F32, tag="h_rl")
                h2_t = ins.tile([P, GJ, D], F32, tag="h_fk")
                p1 = scrp.tile([P, GJ, D], F32, tag="p_rl")
                p2 = scrp.tile([P, GJ, D], F32, tag="p_fk")
                q2 = scrq.tile([P, GJ, D], F32, tag="sq_fk")
                # dummy full-size output for the accum activations (the
                # per-row sums land in stats_q; this tile is never read)
                qd = scrq.tile([P, GJ, D], BF16, tag="sq_dump")
                cols = slice(g * GJ, (g + 1) * GJ)

                js = range(GJ) if g == 0 else (None,)
                for j in js:
                    # g == 0 runs per-row-tile so compute starts after the
                    # first 256 KiB lands instead of after the full 1 MiB
                    jc = slice(None) if j is None else slice(j, j + 1)
                    jl = slice(0, 2) if j is None else slice(j, j + 1)
                    jh = slice(2, GJ) if j is None else None
                    nc.sync.dma_start(out=s_t[:, jc], in_=s_g[g][:, jc])
                    nc.sync.dma_start(out=h1_t[:, jc], in_=hrl_g[g][:, jc])
                    nc.sync.dma_start(out=h2_t[:, jc], in_=hfk_g[g][:, jc])
                    # s*h_rl mult split: Pool takes row-tiles 0-1, DVE 2-3
                    nc.gpsimd.tensor_tensor(
                        out=p1[:, jl], in0=s_t[:, jl], in1=h1_t[:, jl], op=Mul)
                    if jh is not None:
                        nc.vector.tensor_tensor(
                            out=p1[:, jh], in0=s_t[:, jh], in1=h1_t[:, jh],
                            op=Mul)
                    nc.gpsimd.tensor_tensor(
                        out=p2[:, jc], in0=s_t[:, jc], in1=h2_t[:, jc], op=Mul)
                    # norms of s and h_rl: one ACT pass per row-tile each
                    # (squares into a dummy, row-sum into the stats column)
                    for jj in range(GJ) if j is None else (j,):
                        t = g * GJ + jj
                        nc.scalar.activation(
                            out=qd[:, jj], in_=s_t[:, jj], func=Sq,
                            accum_out=ss[:, t: t + 1])
                        nc.scalar.activation(
                            out=qd[:, jj], in_=h1_t[:, jj], func=Sq,
                            accum_out=hh_rl[:, t: t + 1])
                    # norm of h_fk: batched square on ACT + reduce on DVE
                    nc.scalar.activation(out=q2[:, jc], in_=h2_t[:, jc], func=Sq)
                    ct = cols if j is None else slice(g * GJ + j, g * GJ + j + 1)
                    nc.vector.tensor_reduce(out=sp_rl[:, ct], in_=p1[:, jc], **Red)
                    nc.vector.tensor_reduce(out=sp_fk[:, ct], in_=p2[:, jc], **Red)
                    nc.vector.tensor_reduce(out=hh_fk[:, ct], in_=q2[:, jc], **Red)

            # ---- finals on [P, NT] stats tiles ----
            Sqrt = mybir.ActivationFunctionType.Sqrt
            ns = fin.tile([P, NT], F32, tag="ns")
            n1 = fin.tile([P, NT], F32, tag="n1")
            n2 = fin.tile([P, NT], F32, tag="n2")
            nc.scalar.activation(out=ns, in_=ss, func=Sqrt)
            nc.scalar.activation(out=n1, in_=hh_rl, func=Sqrt)
            nc.scalar.activation(out=n2, in_=hh_fk, func=Sqrt)
            nc.vector.tensor_scalar_max(ns, ns, EPS)
            nc.vector.tensor_scalar_max(n1, n1, EPS)
            nc.vector.tensor_scalar_max(n2, n2, EPS)
            den1 = fin.tile([P, NT], F32, tag="den1")
            den2 = fin.tile([P, NT], F32, tag="den2")
            nc.vector.tensor_tensor(den1, ns, n1, op=Mul)
            nc.vector.tensor_tensor(den2, ns, n2, op=Mul)
            nc.vector.reciprocal(den1, den1)
            nc.vector.reciprocal(den2, den2)
            o1 = fin.tile([P, NT], F32, tag="o1")
            o2 = fin.tile([P, NT], F32, tag="o2")
            nc.vector.tensor_tensor(o1, sp_rl, den1, op=Mul)
            nc.vector.tensor_tensor(o2, sp_fk, den2, op=Mul)

            # transpose [P, NT] -> [NT, P] on the (idle) tensor engine
            ident = fin.tile([P, P], F32, tag="ident")
            make_identity(nc, ident)
            po1 = psum.tile([NT, P], F32, tag="po1")
            po2 = psum.tile([NT, P], F32, tag="po2")
            nc.tensor.transpose(po1, o1, ident)
            nc.tensor.transpose(po2, o2, ident)
            o1t = fin.tile([NT, P], F32, tag="o1t")
            o2t = fin.tile([NT, P], F32, tag="o2t")
            nc.scalar.copy(o1t, po1)
            nc.scalar.copy(o2t, po2)
            nc.sync.dma_start(out=out_h[0], in_=o1t)
            nc.sync.dma_start(out=out_h[1], in_=o2t)

    _fix_tail_drain_waits(nc)
    return nc


_NC_CACHE = None


def kernel(s, h_rl, h_fk, trace=False):
    global _NC_CACHE
    s = np.ascontiguousarray(np.asarray(s, dtype=np.float32))
    h_rl = np.ascontiguousarray(np.asarray(h_rl, dtype=np.float32))
    h_fk = np.ascontiguousarray(np.asarray(h_fk, dtype=np.float32))
    assert s.shape == (B, N, D), s.shape

    if _NC_CACHE is None:
        _NC_CACHE = build_nc()
    nc = _NC_CACHE

    in_maps = [
        {"s": s[b], "h_rl": h_rl[b], "h_fk": h_fk[b]} for b in range(B)
    ]
    res = run_bass_kernel_spmd(nc, in_maps, core_ids=list(range(B)), trace=trace)
    out = np.empty((B, 2 * N), dtype=np.float32)
    for b in range(B):
        o = res.results[b]["out"].reshape(2, N)
        out[b, :N] = o[0]
        out[b, N:] = o[1]
    if trace:
        return out, res
    return out
